# revision 18
# baseline (speedup 1.0000x reference)
"""ArcFace loss kernel for 8 Trainium2 NeuronCores.

Strategy (class-parallel, Partial-FC style):
  - weight [100000, 512] is sharded along the class axis: 12500 classes per
    core (padded to 12544 = 7*1792). Shards are passed host-normalized,
    host-transposed ([D, Cpad]) in fp16 so the device streams them straight
    into the TensorEngine as the moving operand.
  - input [512, 512] is normalized and scaled by S on the host, transposed to
    [D, B] fp16, and broadcast to all cores as the stationary operand.
  - Each core computes out[b, c] = <S*in_hat_b, w_hat_c> for its class range
    in natural [B, Cpad] layout: per (super, b-tile) the 4 contraction chunks
    are accumulated into a 4-bank PSUM tile (4 groups of 448 classes), then
    evacuated to fp16 SBUF (VectorE + ScalarE split) and DMA'd out.
  - Stationary = input means only 4 LDWEIGHTS per (super, b-tile) instead of
    one per class chunk, and dummy warm-up matmuls keep the PE p-state
    ramping while the first weight DMAs land.
  - The ArcFace margin only affects one element per row (b, label[b]); the
    host applies the phi transform to those 512 gathered cosines in float64.
"""

import math
import os
import sys

import numpy as np

for _p in ("/opt/trn_rl_repo",):
    if os.path.isdir(_p) and _p not in sys.path:
        sys.path.insert(0, _p)

S = 30.0
MARGIN = 0.5
COS_M = math.cos(MARGIN)
SIN_M = math.sin(MARGIN)
TH = math.cos(math.pi - MARGIN)
MM = math.sin(math.pi - MARGIN) * MARGIN

B, D, C = 512, 512, 100000
NCORES = 8
CSH = C // NCORES            # 12500 classes per core
SUP = 7                      # weight "supers" per core
SUPC = 1792                  # classes per super
CPAD = SUP * SUPC            # 12544
GRP = 4                      # PSUM banks (class groups) per super
GN = SUPC // GRP             # 448 classes per group
BT = B // 128                # 4 batch tiles
DCH = D // 128               # 4 contraction chunks
NWARM = 24                   # PE warm-up matmuls

LAST_RESULT = None
_CACHE = {}


def _build_nc():
    from concourse import bass, bacc, tile, mybir
    from contextlib import ExitStack

    f32 = mybir.dt.float32
    f16 = mybir.dt.float16

    nc = bacc.Bacc()
    xt_e = nc.declare_dram_parameter("xt", [DCH, 128, B], f16, isOutput=False)
    wt_e = nc.declare_dram_parameter("wt", [D, CPAD], f16, isOutput=False)
    out_e = nc.declare_dram_parameter("out", [B, CPAD], f16, isOutput=True)

    with tile.TileContext(nc) as tc, ExitStack() as ctx:
        cpool = ctx.enter_context(tc.tile_pool(name="const", bufs=1))
        xpool = ctx.enter_context(tc.tile_pool(name="xin", bufs=1))
        wpool = ctx.enter_context(tc.tile_pool(name="wts", bufs=2))
        opool = ctx.enter_context(tc.tile_pool(name="outb", bufs=4))
        pm = ctx.enter_context(tc.tile_pool(name="pm", bufs=8, space="PSUM"))

        # warm-up constants (keep the PE busy while the first DMAs land);
        # memset on gpsimd, whose preamble retires earliest
        wstat = cpool.tile([128, 128], f16)
        nc.gpsimd.memset(wstat[:], 1.0)
        wmov = cpool.tile([128, 128], f16)
        nc.gpsimd.memset(wmov[:], 1.0)

        # stationary operand: (S * input_hat).T as [d, b], fp16. Split in
        # two halves interleaved into the sync queue: the first matmul pair
        # (bt 0/1) needs only batch columns 0:256 plus weight chunk d0.
        in_sT = xpool.tile([128, DCH, B], f16)

        warm = pm.tile([128, 512], f32, tag="pm")
        for i in range(NWARM):
            nc.tensor.matmul(
                warm[:, 0:128], wstat[:], wmov[:], start=True, stop=True
            )

        odmas = [nc.scalar.dma_start, nc.gpsimd.dma_start,
                 nc.sync.dma_start]

        for s in range(SUP):
            wts = []
            for d in range(DCH):
                wt_t = wpool.tile([128, SUPC], f16, tag=f"w{d}")
                # even d chunks stream on the sync queue, odd on gpsimd,
                # so the cold start fills from two queues in parallel
                wdma = nc.sync.dma_start if d % 2 == 0 else nc.gpsimd.dma_start
                if s == 0 and d == 0:
                    nc.sync.dma_start(
                        in_sT[:, :, 0:256],
                        xt_e[:, :, 0:256].rearrange("d p b -> p d b"),
                    )
                wdma(
                    wt_t[:],
                    wt_e[d * 128:(d + 1) * 128, s * SUPC:(s + 1) * SUPC],
                )
                if s == 0 and d == 0:
                    nc.sync.dma_start(
                        in_sT[:, :, 256:512],
                        xt_e[:, :, 256:512].rearrange("d p b -> p d b"),
                    )
                wts.append(wt_t)

            if s == 0:
                # cold start: d-major over bt-pairs so all ready matmuls can
                # issue while later weight chunks are still in flight
                for pair in ((0, 1), (2, 3)):
                    pms2 = {}
                    for bt in pair:
                        for g in range(GRP):
                            pms2[(bt, g)] = pm.tile(
                                [128, 512], f32, tag="pm", name=f"pm0_{bt}_{g}"
                            )
                    for d in range(DCH):
                        for bt in pair:
                            stat = in_sT[:, d, bt * 128:(bt + 1) * 128]
                            for g in range(GRP):
                                nc.tensor.matmul(
                                    pms2[(bt, g)][:, 0:GN],
                                    stat,
                                    wts[d][:, g * GN:(g + 1) * GN],
                                    start=(d == 0),
                                    stop=(d == DCH - 1),
                                )
                    for bt in pair:
                        ob = opool.tile([128, SUPC], f16, tag="ob")
                        for g in range(GRP):
                            eng = nc.vector.tensor_copy if g < 2 else nc.scalar.copy
                            eng(ob[:, g * GN:(g + 1) * GN], pms2[(bt, g)][:, 0:GN])
                        odmas[bt % 3](
                            out_e[bt * 128:(bt + 1) * 128, 0:SUPC],
                            ob[:],
                        )
                continue

            for bt in range(BT):
                pms = [
                    pm.tile([128, 512], f32, tag="pm", name=f"pm_{s}_{bt}_{g}")
                    for g in range(GRP)
                ]
                for d in range(DCH):
                    stat = in_sT[:, d, bt * 128:(bt + 1) * 128]
                    for g in range(GRP):
                        nc.tensor.matmul(
                            pms[g][:, 0:GN],
                            stat,
                            wts[d][:, g * GN:(g + 1) * GN],
                            start=(d == 0),
                            stop=(d == DCH - 1),
                        )
                ob = opool.tile([128, SUPC], f16, tag="ob")
                tail = s == SUP - 1
                orows = out_e[bt * 128:(bt + 1) * 128, s * SUPC:(s + 1) * SUPC]
                for g in range(GRP):
                    eng = nc.vector.tensor_copy if g < 2 else nc.scalar.copy
                    eng(ob[:, g * GN:(g + 1) * GN], pms[g][:, 0:GN])
                    if tail and g == 1:
                        odmas[(2 * bt) % 3](orows[:, 0:2 * GN], ob[:, 0:2 * GN])
                if tail:
                    odmas[(2 * bt + 1) % 3](orows[:, 2 * GN:4 * GN], ob[:, 2 * GN:4 * GN])
                else:
                    odmas[(s * BT + bt) % 3](orows, ob[:])
    nc.finalize()
    return nc


def _get_nc():
    if "nc" not in _CACHE:
        _CACHE["nc"] = _build_nc()
    return _CACHE["nc"]


def kernel(input, label, weight):
    global LAST_RESULT
    from concourse.bass_utils import run_bass_kernel_spmd

    inp = np.asarray(input, dtype=np.float32)
    lbl = np.asarray(label).astype(np.int64)
    w = np.asarray(weight, dtype=np.float32)

    # host-side shard prep: normalize, transpose, fp16-cast
    xn = inp / np.maximum(np.linalg.norm(inp, axis=1, keepdims=True), 1e-12)
    xs = np.ascontiguousarray((S * xn).T.astype(np.float16)).reshape(
        DCH, 128, B
    )  # [d, p, b]

    winv = 1.0 / np.maximum(np.linalg.norm(w, axis=1), 1e-12)
    wn = w * winv[:, None]
    wT = np.zeros((NCORES, D, CPAD), dtype=np.float16)
    wT[:, :, :CSH] = wn.reshape(NCORES, CSH, D).transpose(0, 2, 1)

    in_maps = [
        {"xt": xs, "wt": np.ascontiguousarray(wT[k])}
        for k in range(NCORES)
    ]

    nc = _get_nc()
    res = run_bass_kernel_spmd(nc, in_maps, core_ids=list(range(NCORES)))
    LAST_RESULT = res
    outs = res.results

    full = np.empty((B, C), dtype=np.float32)
    for k in range(NCORES):
        blk = np.asarray(outs[k]["out"]).reshape(B, CPAD)[:, :CSH]
        full[:, k * CSH:(k + 1) * CSH] = blk.astype(np.float32)

    # apply the ArcFace margin to the 512 label positions (float64 on host)
    rows = np.arange(B)
    cosl = np.clip(full[rows, lbl].astype(np.float64) / S, -1.0, 1.0)
    sine = np.sqrt(np.clip(1.0 - cosl * cosl, 1e-9, 1.0))
    phi = cosl * COS_M - sine * SIN_M
    phi = np.where(cosl > TH, phi, cosl - MM)
    full[rows, lbl] = (S * phi).astype(np.float32)
    return full


# revision 23
# speedup vs baseline: 1.0145x; 1.0145x over previous
"""ArcFace loss kernel for 8 Trainium2 NeuronCores.

Strategy (class-parallel, Partial-FC style):
  - weight [100000, 512] is sharded along the class axis: 12500 classes per
    core (padded to 12544 = 7*1792). Shards are passed host-normalized,
    host-transposed ([D, Cpad]) in fp16 so the device streams them straight
    into the TensorEngine as the moving operand.
  - input [512, 512] is normalized and scaled by S on the host, transposed to
    [D, B] fp16, and broadcast to all cores as the stationary operand.
  - Each core computes out[b, c] = <S*in_hat_b, w_hat_c> for its class range
    in natural [B, Cpad] layout: per (super, b-tile) the 4 contraction chunks
    are accumulated into a 4-bank PSUM tile (4 groups of 448 classes), then
    evacuated to fp16 SBUF (VectorE + ScalarE split) and DMA'd out.
  - Stationary = input means only 4 LDWEIGHTS per (super, b-tile) instead of
    one per class chunk, and dummy warm-up matmuls keep the PE p-state
    ramping while the first weight DMAs land.
  - The ArcFace margin only affects one element per row (b, label[b]); the
    host applies the phi transform to those 512 gathered cosines in float64.
"""

import math
import os
import sys

import numpy as np

for _p in ("/opt/trn_rl_repo",):
    if os.path.isdir(_p) and _p not in sys.path:
        sys.path.insert(0, _p)

S = 30.0
MARGIN = 0.5
COS_M = math.cos(MARGIN)
SIN_M = math.sin(MARGIN)
TH = math.cos(math.pi - MARGIN)
MM = math.sin(math.pi - MARGIN) * MARGIN

B, D, C = 512, 512, 100000
NCORES = 8
CSH = C // NCORES            # 12500 classes per core
SUP = 7                      # weight "supers" per core
SUPC = 1792                  # classes per super
CPAD = SUP * SUPC            # 12544
GRP = 4                      # PSUM banks (class groups) per super
GN = SUPC // GRP             # 448 classes per group
BT = B // 128                # 4 batch tiles
DCH = D // 128               # 4 contraction chunks
NWARM = 32                   # PE warm-up matmuls

LAST_RESULT = None
_CACHE = {}


def _build_nc():
    from concourse import bass, bacc, tile, mybir
    from contextlib import ExitStack

    f32 = mybir.dt.float32
    f16 = mybir.dt.float16

    nc = bacc.Bacc()
    xt_e = nc.declare_dram_parameter("xt", [128, DCH, B], f16, isOutput=False)
    wt_e = nc.declare_dram_parameter("wt", [D, CPAD], f16, isOutput=False)
    out_e = nc.declare_dram_parameter("out", [B, CPAD], f16, isOutput=True)

    with tile.TileContext(nc) as tc, ExitStack() as ctx:
        cpool = ctx.enter_context(tc.tile_pool(name="const", bufs=1))
        xpool = ctx.enter_context(tc.tile_pool(name="xin", bufs=1))
        wpool = ctx.enter_context(tc.tile_pool(name="wts", bufs=2))
        opool = ctx.enter_context(tc.tile_pool(name="outb", bufs=4))
        pm = ctx.enter_context(tc.tile_pool(name="pm", bufs=8, space="PSUM"))

        # warm-up constants (keep the PE busy while the first DMAs land);
        # memset on gpsimd, whose preamble retires earliest
        wstat = cpool.tile([128, 128], f16)
        nc.gpsimd.memset(wstat[:], 1.0)
        wmov = cpool.tile([128, 128], f16)
        nc.gpsimd.memset(wmov[:], 1.0)

        # stationary operand: (S * input_hat).T, host pre-shuffled into the
        # SBUF layout [p, d, b] so one DMA moves 4KB-contiguous rows; it
        # rides the gpsimd queue, parallel to the sync-queue weight stream
        in_sT = xpool.tile([128, DCH, B], f16)
        nc.gpsimd.dma_start(in_sT[:], xt_e[:, :, :])

        warm = pm.tile([128, 512], f32, tag="pm")
        for i in range(NWARM):
            nc.tensor.matmul(
                warm[:, 0:128], wstat[:], wmov[:], start=True, stop=True
            )

        odmas = [nc.scalar.dma_start, nc.gpsimd.dma_start,
                 nc.sync.dma_start]

        for s in range(SUP):
            wts = []
            for d in range(DCH):
                wt_t = wpool.tile([128, SUPC], f16, tag=f"w{d}")
                nc.sync.dma_start(
                    wt_t[:],
                    wt_e[d * 128:(d + 1) * 128, s * SUPC:(s + 1) * SUPC],
                )
                wts.append(wt_t)

            if s == 0:
                # cold start: d-major over bt-pairs so all ready matmuls can
                # issue while later weight chunks are still in flight
                for pair in ((0, 1), (2, 3)):
                    pms2 = {}
                    for bt in pair:
                        for g in range(GRP):
                            pms2[(bt, g)] = pm.tile(
                                [128, 512], f32, tag="pm", name=f"pm0_{bt}_{g}"
                            )
                    for d in range(DCH):
                        for bt in pair:
                            stat = in_sT[:, d, bt * 128:(bt + 1) * 128]
                            for g in range(GRP):
                                nc.tensor.matmul(
                                    pms2[(bt, g)][:, 0:GN],
                                    stat,
                                    wts[d][:, g * GN:(g + 1) * GN],
                                    start=(d == 0),
                                    stop=(d == DCH - 1),
                                )
                    for bt in pair:
                        ob = opool.tile([128, SUPC], f16, tag="ob")
                        for g in range(GRP):
                            eng = nc.vector.tensor_copy if g < 2 else nc.scalar.copy
                            eng(ob[:, g * GN:(g + 1) * GN], pms2[(bt, g)][:, 0:GN])
                        odmas[bt % 3](
                            out_e[bt * 128:(bt + 1) * 128, 0:SUPC],
                            ob[:],
                        )
                continue

            for bt in range(BT):
                pms = [
                    pm.tile([128, 512], f32, tag="pm", name=f"pm_{s}_{bt}_{g}")
                    for g in range(GRP)
                ]
                for d in range(DCH):
                    stat = in_sT[:, d, bt * 128:(bt + 1) * 128]
                    for g in range(GRP):
                        nc.tensor.matmul(
                            pms[g][:, 0:GN],
                            stat,
                            wts[d][:, g * GN:(g + 1) * GN],
                            start=(d == 0),
                            stop=(d == DCH - 1),
                        )
                ob = opool.tile([128, SUPC], f16, tag="ob")
                tail = s == SUP - 1
                orows = out_e[bt * 128:(bt + 1) * 128, s * SUPC:(s + 1) * SUPC]
                for g in range(GRP):
                    eng = nc.vector.tensor_copy if g < 2 else nc.scalar.copy
                    eng(ob[:, g * GN:(g + 1) * GN], pms[g][:, 0:GN])
                    if tail and g == 1:
                        odmas[(2 * bt) % 3](orows[:, 0:2 * GN], ob[:, 0:2 * GN])
                if tail:
                    odmas[(2 * bt + 1) % 3](orows[:, 2 * GN:4 * GN], ob[:, 2 * GN:4 * GN])
                else:
                    odmas[(s * BT + bt) % 3](orows, ob[:])
    nc.finalize()
    return nc


def _get_nc():
    if "nc" not in _CACHE:
        _CACHE["nc"] = _build_nc()
    return _CACHE["nc"]


def kernel(input, label, weight):
    global LAST_RESULT
    from concourse.bass_utils import run_bass_kernel_spmd

    inp = np.asarray(input, dtype=np.float32)
    lbl = np.asarray(label).astype(np.int64)
    w = np.asarray(weight, dtype=np.float32)

    # host-side shard prep: normalize, transpose, fp16-cast
    xn = inp / np.maximum(np.linalg.norm(inp, axis=1, keepdims=True), 1e-12)
    xs = np.ascontiguousarray(
        (S * xn).T.astype(np.float16).reshape(DCH, 128, B).transpose(1, 0, 2)
    )  # [p, d, b] — matches the in_sT SBUF layout

    winv = 1.0 / np.maximum(np.linalg.norm(w, axis=1), 1e-12)
    wn = w * winv[:, None]
    wT = np.zeros((NCORES, D, CPAD), dtype=np.float16)
    wT[:, :, :CSH] = wn.reshape(NCORES, CSH, D).transpose(0, 2, 1)

    in_maps = [
        {"xt": xs, "wt": np.ascontiguousarray(wT[k])}
        for k in range(NCORES)
    ]

    nc = _get_nc()
    res = run_bass_kernel_spmd(nc, in_maps, core_ids=list(range(NCORES)))
    LAST_RESULT = res
    outs = res.results

    full = np.empty((B, C), dtype=np.float32)
    for k in range(NCORES):
        blk = np.asarray(outs[k]["out"]).reshape(B, CPAD)[:, :CSH]
        full[:, k * CSH:(k + 1) * CSH] = blk.astype(np.float32)

    # apply the ArcFace margin to the 512 label positions (float64 on host)
    rows = np.arange(B)
    cosl = np.clip(full[rows, lbl].astype(np.float64) / S, -1.0, 1.0)
    sine = np.sqrt(np.clip(1.0 - cosl * cosl, 1e-9, 1.0))
    phi = cosl * COS_M - sine * SIN_M
    phi = np.where(cosl > TH, phi, cosl - MM)
    full[rows, lbl] = (S * phi).astype(np.float32)
    return full


# revision 28
# speedup vs baseline: 1.0310x; 1.0163x over previous
"""ArcFace loss kernel for 8 Trainium2 NeuronCores.

Strategy (class-parallel, Partial-FC style):
  - weight [100000, 512] is sharded along the class axis: 12500 classes per
    core (padded to 12544 = 7*1792). Shards are passed host-normalized,
    host-transposed ([D, Cpad]) in fp16 so the device streams them straight
    into the TensorEngine as the moving operand.
  - input [512, 512] is normalized and scaled by S on the host, transposed to
    [D, B] fp16, and broadcast to all cores as the stationary operand.
  - Each core computes out[b, c] = <S*in_hat_b, w_hat_c> for its class range
    in natural [B, Cpad] layout: per (super, b-tile) the 4 contraction chunks
    are accumulated into a 4-bank PSUM tile (4 groups of 448 classes), then
    evacuated to fp16 SBUF (VectorE + ScalarE split) and DMA'd out.
  - Stationary = input means only 4 LDWEIGHTS per (super, b-tile) instead of
    one per class chunk, and dummy warm-up matmuls keep the PE p-state
    ramping while the first weight DMAs land.
  - The ArcFace margin only affects one element per row (b, label[b]); the
    host applies the phi transform to those 512 gathered cosines in float64.
"""

import math
import os
import sys

import numpy as np

for _p in ("/opt/trn_rl_repo",):
    if os.path.isdir(_p) and _p not in sys.path:
        sys.path.insert(0, _p)

S = 30.0
MARGIN = 0.5
COS_M = math.cos(MARGIN)
SIN_M = math.sin(MARGIN)
TH = math.cos(math.pi - MARGIN)
MM = math.sin(math.pi - MARGIN) * MARGIN

B, D, C = 512, 512, 100000
NCORES = 8
CSH = C // NCORES            # 12500 classes per core
SUP = 7                      # weight "supers" per core
SUPC = 1792                  # classes per super
CPAD = SUP * SUPC            # 12544
GRP = 4                      # PSUM banks (class groups) per super
GN = SUPC // GRP             # 448 classes per group
BT = B // 128                # 4 batch tiles
DCH = D // 128               # 4 contraction chunks
NWARM = 32                   # PE warm-up matmuls

LAST_RESULT = None
_CACHE = {}


def _build_nc():
    from concourse import bass, bacc, tile, mybir
    from contextlib import ExitStack

    f32 = mybir.dt.float32
    f16 = mybir.dt.float16

    nc = bacc.Bacc()
    xt_e = nc.declare_dram_parameter("xt", [128, 2, DCH, 256], f16, isOutput=False)
    wt_e = nc.declare_dram_parameter("wt", [D, CPAD], f16, isOutput=False)
    out_e = nc.declare_dram_parameter("out", [B, CPAD], f16, isOutput=True)

    with tile.TileContext(nc) as tc, ExitStack() as ctx:
        cpool = ctx.enter_context(tc.tile_pool(name="const", bufs=1))
        xpool = ctx.enter_context(tc.tile_pool(name="xin", bufs=1))
        wpool = ctx.enter_context(tc.tile_pool(name="wts", bufs=2))
        opool = ctx.enter_context(tc.tile_pool(name="outb", bufs=4))
        pm = ctx.enter_context(tc.tile_pool(name="pm", bufs=8, space="PSUM"))

        # warm-up constants (keep the PE busy while the first DMAs land);
        # memset on gpsimd, whose preamble retires earliest
        wstat = cpool.tile([128, 128], f16)
        nc.gpsimd.memset(wstat[:], 1.0)
        wmov = cpool.tile([128, 128], f16)
        nc.gpsimd.memset(wmov[:], 1.0)

        # stationary operand: (S * input_hat).T, host pre-shuffled into the
        # SBUF layout [p, h, d, b%256]. Everything latency-critical rides
        # the sync queue (the gpsimd/scalar DMA queues are ~4x slower):
        # first batch-half, first weight chunk, second half, rest.
        in_sT = xpool.tile([128, 2, DCH, 256], f16)

        warm = pm.tile([128, 512], f32, tag="pm")
        for i in range(NWARM):
            nc.tensor.matmul(
                warm[:, 0:128], wstat[:], wmov[:], start=True, stop=True
            )

        odmas = [nc.scalar.dma_start, nc.gpsimd.dma_start,
                 nc.sync.dma_start]

        def stat_ap(d, bt):
            h, j = divmod(bt, 2)
            return in_sT[:, h, d, j * 128:(j + 1) * 128]

        for s in range(SUP):
            wts = []
            for d in range(DCH):
                wt_t = wpool.tile([128, SUPC], f16, tag=f"w{d}")
                if s == 0 and d == 0:
                    nc.sync.dma_start(in_sT[:, 0], xt_e[:, 0])
                nc.sync.dma_start(
                    wt_t[:],
                    wt_e[d * 128:(d + 1) * 128, s * SUPC:(s + 1) * SUPC],
                )
                if s == 0 and d == 0:
                    nc.sync.dma_start(in_sT[:, 1], xt_e[:, 1])
                wts.append(wt_t)

            if s == 0:
                # cold start: d-major over bt-pairs so all ready matmuls can
                # issue while later weight chunks are still in flight
                for pair in ((0, 1), (2, 3)):
                    pms2 = {}
                    for bt in pair:
                        for g in range(GRP):
                            pms2[(bt, g)] = pm.tile(
                                [128, 512], f32, tag="pm", name=f"pm0_{bt}_{g}"
                            )
                    for d in range(DCH):
                        for bt in pair:
                            stat = stat_ap(d, bt)
                            for g in range(GRP):
                                nc.tensor.matmul(
                                    pms2[(bt, g)][:, 0:GN],
                                    stat,
                                    wts[d][:, g * GN:(g + 1) * GN],
                                    start=(d == 0),
                                    stop=(d == DCH - 1),
                                )
                    for bt in pair:
                        ob = opool.tile([128, SUPC], f16, tag="ob")
                        for g in range(GRP):
                            eng = nc.vector.tensor_copy if g < 2 else nc.scalar.copy
                            eng(ob[:, g * GN:(g + 1) * GN], pms2[(bt, g)][:, 0:GN])
                        odmas[bt % 2](
                            out_e[bt * 128:(bt + 1) * 128, 0:SUPC],
                            ob[:],
                        )
                continue

            for bt in range(BT):
                pms = [
                    pm.tile([128, 512], f32, tag="pm", name=f"pm_{s}_{bt}_{g}")
                    for g in range(GRP)
                ]
                for d in range(DCH):
                    stat = stat_ap(d, bt)
                    for g in range(GRP):
                        nc.tensor.matmul(
                            pms[g][:, 0:GN],
                            stat,
                            wts[d][:, g * GN:(g + 1) * GN],
                            start=(d == 0),
                            stop=(d == DCH - 1),
                        )
                ob = opool.tile([128, SUPC], f16, tag="ob")
                tail = s == SUP - 1
                orows = out_e[bt * 128:(bt + 1) * 128, s * SUPC:(s + 1) * SUPC]
                for g in range(GRP):
                    eng = nc.vector.tensor_copy if g < 2 else nc.scalar.copy
                    eng(ob[:, g * GN:(g + 1) * GN], pms[g][:, 0:GN])
                    if tail and g == 1:
                        nc.sync.dma_start(orows[:, 0:2 * GN], ob[:, 0:2 * GN])
                if tail:
                    nc.sync.dma_start(orows[:, 2 * GN:4 * GN], ob[:, 2 * GN:4 * GN])
                elif s == SUP - 2:
                    nc.sync.dma_start(orows, ob[:])
                else:
                    odmas[(s * BT + bt) % 2](orows, ob[:])
    nc.finalize()
    return nc


def _get_nc():
    if "nc" not in _CACHE:
        _CACHE["nc"] = _build_nc()
    return _CACHE["nc"]


def kernel(input, label, weight):
    global LAST_RESULT
    from concourse.bass_utils import run_bass_kernel_spmd

    inp = np.asarray(input, dtype=np.float32)
    lbl = np.asarray(label).astype(np.int64)
    w = np.asarray(weight, dtype=np.float32)

    # host-side shard prep: normalize, transpose, fp16-cast
    xn = inp / np.maximum(np.linalg.norm(inp, axis=1, keepdims=True), 1e-12)
    xs = np.ascontiguousarray(
        (S * xn).T.astype(np.float16)
        .reshape(DCH, 128, 2, 256)
        .transpose(1, 2, 0, 3)
    )  # [p, h, d, b%256] — matches the in_sT SBUF layout

    winv = 1.0 / np.maximum(np.linalg.norm(w, axis=1), 1e-12)
    wn = w * winv[:, None]
    wT = np.zeros((NCORES, D, CPAD), dtype=np.float16)
    wT[:, :, :CSH] = wn.reshape(NCORES, CSH, D).transpose(0, 2, 1)

    in_maps = [
        {"xt": xs, "wt": np.ascontiguousarray(wT[k])}
        for k in range(NCORES)
    ]

    nc = _get_nc()
    res = run_bass_kernel_spmd(nc, in_maps, core_ids=list(range(NCORES)))
    LAST_RESULT = res
    outs = res.results

    full = np.empty((B, C), dtype=np.float32)
    for k in range(NCORES):
        blk = np.asarray(outs[k]["out"]).reshape(B, CPAD)[:, :CSH]
        full[:, k * CSH:(k + 1) * CSH] = blk.astype(np.float32)

    # apply the ArcFace margin to the 512 label positions (float64 on host)
    rows = np.arange(B)
    cosl = np.clip(full[rows, lbl].astype(np.float64) / S, -1.0, 1.0)
    sine = np.sqrt(np.clip(1.0 - cosl * cosl, 1e-9, 1.0))
    phi = cosl * COS_M - sine * SIN_M
    phi = np.where(cosl > TH, phi, cosl - MM)
    full[rows, lbl] = (S * phi).astype(np.float32)
    return full
